# revision 12
# baseline (speedup 1.0000x reference)
"""Cross-attention kernel for Trainium2 (8 NeuronCores, batch-parallel).

Math per batch b (reference semantics):
  q = queries[b].reshape(C, N).T + q_pos        # [N, C]
  k = keys[b].reshape(C, N).T + k_pos
  v = values[b].reshape(C, N).T                 # [N, C]
  out = softmax(q @ k.T / 16) @ v               # [N, Cv]

Device layout (per core = one batch):
  S is computed transposed (S^T[k, q]) so exp(S^T) tiles are directly the
  STATIONARY operand of the O matmul (O[q, c] = sum_k A^T[k, q]^T V[k, c]).
  V chunks are augmented with two ones columns (f32r matmuls need an even
  moving free dim) so the softmax denominator accumulates in PSUM columns
  C/C+1 for free; the final normalization is a per-partition reciprocal +
  scalar multiply on the (otherwise idle) vector engine, staggered into the
  next block's key loop so it never clogs the activation queue.

  Q/K have the position embeddings folded in host-side and are split into
  fp8e4m3 hi+lo halves; S = Kh Qh + Kh Ql + Kl Qh runs as 256-deep
  DoubleRow fp8 matmuls (0.5 PE cycles/row -- 2x the f32r rate).  The
  dropped Kl Ql term and residual quantization contribute ~1e-3 relative
  error on the logits.  The O matmul stays f32r (A stationary, V moving).
"""

import numpy as np

import concourse.bass as bass
import concourse.tile as tile
import concourse.mybir as mybir
from concourse import bacc
from concourse.bass_utils import run_bass_kernel_spmd

P = 128          # partitions
C = 256          # qk/v channel dim
CA = C + 2       # v width augmented with ones columns (must be even)
N = 4096         # sequence (64*64)
B = 8            # batch == n_cores
QW = 512         # query block width (max matmul moving free dim)
NQB = N // QW    # 8 query blocks
NKO = N // P     # 32 key chunks
KPB = QW // P    # key chunks per K block tile
VB = 4           # v chunks loaded per DMA
LAG = 6          # O-matmul lag behind exp, in key chunks
SCALE = 1.0 / 16.0  # 1/sqrt(C)

F32 = mybir.dt.float32
F32R = mybir.dt.float32r
F8 = mybir.dt.float8e4
AF = mybir.ActivationFunctionType
DR = mybir.MatmulPerfMode.DoubleRow

_NC_CACHE = None


def tf32_round(x: np.ndarray) -> np.ndarray:
    u = x.view(np.uint32)
    u = (u + np.uint32(0x1000)) & np.uint32(0xFFFFE000)
    return u.view(np.float32)


def build_nc(ps_s_bufs=4, po_bufs=4, lag=LAG, n_warm=4, atp_bufs=None):
    atp_bufs = (lag + 4) if atp_bufs is None else atp_bufs
    nc = bacc.Bacc(None, target_bir_lowering=False)
    q8 = nc.dram_tensor("q8", [2, C, N], F8, kind="ExternalInput")
    k8 = nc.dram_tensor("k8", [2, C, N], F8, kind="ExternalInput")
    v = nc.dram_tensor("v", [N, C], F32R, kind="ExternalInput")
    o = nc.dram_tensor("o", [N, C], F32, kind="ExternalOutput")

    q84 = q8.rearrange("hl (co p) n -> p hl co n", p=P)
    k84 = k8.rearrange("hl (co p) n -> p hl co n", p=P)
    v3 = v.rearrange("(g p) c -> p g c", p=P)
    o3 = o.rearrange("(nb p) c -> p nb c", p=P)

    with tile.TileContext(nc) as tc:
        with (
            tc.tile_pool(name="consts", bufs=1) as consts,
            tc.tile_pool(name="kk", bufs=NQB) as kk,
            tc.tile_pool(name="qq", bufs=2) as qq,
            tc.tile_pool(name="vp", bufs=NKO // VB) as vp,
            tc.tile_pool(name="atp", bufs=atp_bufs) as atp,
            tc.tile_pool(name="small", bufs=8) as small,
            tc.tile_pool(name="outp", bufs=2) as outp,
            tc.tile_pool(name="ps_s", bufs=ps_s_bufs, space="PSUM") as ps_s,
            tc.tile_pool(name="ps_o", bufs=po_bufs, space="PSUM") as ps_o,
        ):
            warm_f = consts.tile([P, QW], F32, tag="warm_f")
            nc.vector.memset(warm_f, 1.0)
            warm = consts.tile([P, QW], F32R, tag="warm")
            nc.vector.tensor_copy(warm, warm_f)
            ones_f = consts.tile([P, VB, CA - C], F32, tag="ones_f")
            nc.vector.memset(ones_f, 1.0)

            def load_kblk(j, split=False):
                sl = slice(j * QW, (j + 1) * QW)
                kb = kk.tile([P, 2, 2, QW], F8, tag="k8")
                if split:
                    nc.sync.dma_start(kb[:, :, :, 0:P], k84[:, :, :, 0:P])
                    nc.sync.dma_start(kb[:, :, :, P:QW],
                                      k84[:, :, :, j * QW + P : (j + 1) * QW])
                else:
                    nc.sync.dma_start(kb, k84[:, :, :, sl])
                return kb

            def load_vgroup(g):
                vg = vp.tile([P, VB, CA], F32R, tag="v")
                nc.sync.dma_start(vg[:, :, 0:C], v3[:, g * VB : (g + 1) * VB, :])
                nc.vector.tensor_copy(vg[:, :, C:CA], ones_f)
                return vg

            kblks = {}
            vgs = {}

            def epilogue_piece(j, po, ob, qs, on_act=False, eager_dma=False):
                inv = small.tile([P, 1], F32, tag="inv")
                nc.vector.reciprocal(inv, po[qs][:, C : C + 1])
                if on_act:
                    nc.scalar.activation(ob[:, qs, :], po[qs][:, 0:C],
                                         AF.Copy, scale=inv)
                else:
                    nc.vector.tensor_scalar_mul(ob[:, qs, :], po[qs][:, 0:C],
                                                inv)
                if eager_dma:
                    nc.sync.dma_start(o3[:, 4 * j + qs, :], ob[:, qs, :])
                elif qs == 3:
                    nc.sync.dma_start(o3[:, 4 * j : 4 * j + 4, :], ob)

            pending = None
            for j in range(NQB):
                sl = slice(j * QW, (j + 1) * QW)
                qb = qq.tile([P, 2, 2, QW], F8, tag="q8")
                if j == 0:
                    nc.sync.dma_start(qb[:, 0, :, :], q84[:, 0, :, sl])
                    kb0 = kk.tile([P, 2, 2, QW], F8, tag="k8", name="kb0")
                    nc.sync.dma_start(kb0[:, :, :, 0:P], k84[:, :, :, 0:P])
                    nc.sync.dma_start(qb[:, 1, :, :], q84[:, 1, :, sl])
                    nc.sync.dma_start(kb0[:, :, :, P:QW], k84[:, :, :, P:QW])
                    kblks[0] = kb0
                    # p-state warm-up: keep PE busy while the first loads fly
                    wps = ps_s.tile([P, QW], F32, tag="s", name="wps")
                    for w in range(n_warm):
                        nc.tensor.matmul(wps, warm[:, 0:P], warm,
                                         start=True, stop=True)
                else:
                    nc.sync.dma_start(qb, q84[:, :, :, sl])

                if j == 0:
                    # deadline-ordered remaining loads: K(jb+1) then V(jb);
                    # the first v group is split so chunk 0 arrives sooner
                    for jb in range(NQB):
                        if jb + 1 < NQB:
                            kblks[jb + 1] = load_kblk(jb + 1)
                        if jb == 0:
                            vg = vp.tile([P, VB, CA], F32R, tag="v", name="vg0")
                            nc.sync.dma_start(vg[:, 0:2, 0:C], v3[:, 0:2, :])
                            nc.sync.dma_start(vg[:, 2:4, 0:C], v3[:, 2:4, :])
                            nc.vector.tensor_copy(vg[:, :, C:CA], ones_f)
                            vgs[0] = vg
                        else:
                            vgs[jb] = load_vgroup(jb)

                po = [ps_o.tile([P, CA], F32, tag="po", name=f"po{qs}",
                                padded_shape=[P, QW]) for qs in range(4)]

                a_q = {}

                def o_matmuls(ko):
                    av = a_q[ko]
                    vc = vgs[ko // VB][:, ko % VB, :]
                    for qs in range(4):
                        nc.tensor.matmul(
                            po[qs],
                            av[:, qs * P : (qs + 1) * P],
                            vc,
                            start=(ko == 0),
                            stop=(ko == NKO - 1),
                        )
                    del a_q[ko]

                for ko in range(NKO):
                    pss = ps_s.tile([P, QW], F32, tag="s")
                    jb, koff = divmod(ko, KPB)
                    ksl = slice(koff * P, (koff + 1) * P)
                    kb = kblks[jb]
                    nc.tensor.matmul(pss, kb[:, 0, :, ksl], qb[:, 0, :, :],
                                     start=True, stop=False, perf_mode=DR)
                    nc.tensor.matmul(pss, kb[:, 0, :, ksl], qb[:, 1, :, :],
                                     start=False, stop=False, perf_mode=DR)
                    nc.tensor.matmul(pss, kb[:, 1, :, ksl], qb[:, 0, :, :],
                                     start=False, stop=True, perf_mode=DR)
                    a = atp.tile([P, QW], F32R, tag="a")
                    nc.scalar.activation(a, pss, AF.Exp, scale=SCALE)
                    a_q[ko] = a

                    if pending is not None and ko < 4:
                        if ko == 0:
                            ob = outp.tile([P, 4, C], F32, tag="ot")
                            pending = (*pending, ob)
                        epilogue_piece(pending[0], pending[1], pending[2], ko)
                        if ko == 3:
                            pending = None

                    if ko >= lag:
                        o_matmuls(ko - lag)

                if j < NQB - 1:
                    for ko in range(NKO - lag, NKO):
                        o_matmuls(ko)
                    pending = (j, po)
                else:
                    # final block: drain qs-major so each accumulation group
                    # closes early and its epilogue+store pipelines behind
                    # the remaining matmuls
                    ob = outp.tile([P, 4, C], F32, tag="ot")
                    for qs in range(4):
                        for ko in range(NKO - lag, NKO):
                            nc.tensor.matmul(
                                po[qs],
                                a_q[ko][:, qs * P : (qs + 1) * P],
                                vgs[ko // VB][:, ko % VB, :],
                                start=False,
                                stop=(ko == NKO - 1),
                            )
                        epilogue_piece(j, po, ob, qs, on_act=(qs % 2 == 1),
                                       eager_dma=True)

    nc.compile()
    return nc


def _get_nc():
    global _NC_CACHE
    if _NC_CACHE is None:
        _NC_CACHE = build_nc()
    return _NC_CACHE


def make_in_maps(queries, keys, values, q_pos_embedding, k_pos_embedding):
    queries = np.asarray(queries, dtype=np.float32)
    keys = np.asarray(keys, dtype=np.float32)
    values = np.asarray(values, dtype=np.float32)
    fp8 = mybir.dt.np(F8)
    qpT = np.asarray(q_pos_embedding, dtype=np.float32).reshape(N, C).T
    kpT = np.asarray(k_pos_embedding, dtype=np.float32).reshape(N, C).T
    in_maps = []
    for b in range(B):
        qt = queries[b].reshape(C, N) + qpT
        kt = keys[b].reshape(C, N) + kpT
        qh8 = qt.astype(fp8)
        ql8 = (qt - qh8.astype(np.float32)).astype(fp8)
        kh8 = kt.astype(fp8)
        kl8 = (kt - kh8.astype(np.float32)).astype(fp8)
        vT = tf32_round(np.ascontiguousarray(values[b].reshape(C, N).T))
        in_maps.append({
            "q8": np.ascontiguousarray(np.stack([qh8, ql8])),
            "k8": np.ascontiguousarray(np.stack([kh8, kl8])),
            "v": vT,
        })
    return in_maps


def kernel(queries, keys, values, q_pos_embedding, k_pos_embedding):
    nc = _get_nc()
    in_maps = make_in_maps(queries, keys, values, q_pos_embedding,
                           k_pos_embedding)
    res = run_bass_kernel_spmd(nc, in_maps, core_ids=list(range(B)))
    out = np.stack([r["o"].T.reshape(C, 64, 64) for r in res.results])
    return out.astype(np.float32)
